# revision 21
# baseline (speedup 1.0000x reference)
"""Cosformer attention (causal linear attention with cos reweighting) on 8
Trainium2 NeuronCores.

Sharding: n = bsz*heads = 16 sequences -> 2 per core. Core c handles batch-half
i = c//4 and head-pair p = c%4 (heads 2p, 2p+1). Fully data/head parallel; the
only cross-core interaction is the host-side sum of output-projection partials.

v4 design notes:
  - All DMA images bf16; raw (non-duplicated) pair projections (24 matmuls);
    [sin;cos] feature expansion via DVE elementwise ops against two on-device
    tables (scb=[sin;cos] rows, scbf=[cos;sin] rows so inputs always share a
    base partition; outputs may be partition-shifted).
  - Pair-merged single-bank PSUM tiles: scores (128,256), qkv (128,130),
    S-state (128,130, persistent): one start=True clears the bank, the other
    region writes use start=False (has_written=0 -> overwrite).
  - Normalize: one strided ts_max over both denominators + two ts_divide
    (per-partition divisor) straight out of PSUM.
  - attn^T for the output projection comes from an SBUF->SBUF DMA transpose
    on the idle sync ring (no PE transpose, no PSUM, no ACT copy).
  - th1 projections are interleaved between attention chunks 0-3 to keep the
    PE dense (HAM stays at full clock) and avoid a serial projection wall.
  - Output (128, 4096) bf16 chunk-major; 4 HWDGE stores (2 chunks each).
"""

import os
import sys

import numpy as np

for _p in ("/opt/trn_rl_repo", "/root/.axon_site/_ro/trn_rl_repo"):
    if os.path.isdir(_p) and _p not in sys.path:
        sys.path.insert(0, _p)

N_HEAD = 8
E = 512
L = 1024  # sequence length per batch-half
BSZ = 2
D = 64  # head dim
P = 128  # partition/chunk/pair-feature size
NCHUNK = L // P
EPS = 1e-6
N_CORES = 8
TH = 512  # token-half width for projections

# c1: [biases as fp32-bits (4 bf16 cols: bq_pair, bk_pair) | wq_pair | wk_pair]
# loaded as two micro-packs: c1[:, 0:516] (bias+wq) on sync, c1[:, 516:1028]
# (wk) on scalar, so the first projection can start as early as possible.
_C1_BIAS = 0
_C1_WQ = 4
_C1_WK = 516
_C1_COLS = 1028
# c3: [wv (512) | wo (512) | mask_pair (256) | ident (128)]
_C3_WV = 0
_C3_WO = 512
_C3_MASK = 1024
_C3_IDENT = 1280
_C3_COLS = 1408

_CACHE = {}


def _build_bass():
    import concourse.bass as bass
    import concourse.tile as tile
    from concourse import bacc, mybir
    from contextlib import ExitStack

    f32 = mybir.dt.float32
    i32 = mybir.dt.int32
    bf16 = mybir.dt.bfloat16
    AF = mybir.ActivationFunctionType
    ALU = mybir.AluOpType

    nc = bacc.Bacc("TRN2", target_bir_lowering=False, debug=False)

    xt_d = nc.dram_tensor("xt", [P, 4096], bf16, kind="ExternalInput")
    c1_d = nc.dram_tensor("c1", [P, _C1_COLS], bf16, kind="ExternalInput")
    c3_d = nc.dram_tensor("c3", [P, _C3_COLS], bf16, kind="ExternalInput")
    out_d = nc.dram_tensor("out", [P, 4096], bf16, kind="ExternalOutput")

    with tile.TileContext(nc) as tc:
        with ExitStack() as ctx:
            ep = ctx.enter_context
            cpool = ep(tc.tile_pool(name="const", bufs=1))
            seqp = ep(tc.tile_pool(name="seq", bufs=1))
            ktokp = ep(tc.tile_pool(name="ktok", bufs=8))
            bp = ep(tc.tile_pool(name="bsb", bufs=4))
            sp = ep(tc.tile_pool(name="state", bufs=4))
            app = ep(tc.tile_pool(name="apair", bufs=2))
            atp = ep(tc.tile_pool(name="attnT", bufs=2))
            outp = ep(tc.tile_pool(name="outsb", bufs=2))
            rp = ep(tc.tile_pool(name="rcol", bufs=4))
            big_ps = ep(tc.tile_pool(name="bigps", bufs=2, space="PSUM"))
            sq_ps = ep(tc.tile_pool(name="sqps", bufs=3, space="PSUM"))
            acc_ps = ep(tc.tile_pool(name="accps", bufs=2, space="PSUM"))
            s_ps = ep(tc.tile_pool(name="sps", bufs=1, space="PSUM"))

            # ---- loads: two HWDGE rings, first-needed bytes first.
            # sync ring then goes quiet early so the attn-phase at/kt
            # DMA-transposes never queue behind bulk traffic; all output
            # stores go on the scalar ring.
            c1_t = cpool.tile([P, _C1_COLS], bf16, name="c1_t")
            xt_sb = cpool.tile([P, 4096], bf16, name="xt_sb")
            nc.sync.dma_start(c1_t[:, 0:516], c1_d[:, 0:516])
            nc.scalar.dma_start(c1_t[:, 516:1028], c1_d[:, 516:1028])
            nc.sync.dma_start(xt_sb[:, 0:512], xt_d[:, 0:512])
            nc.scalar.dma_start(xt_sb[:, 1024:1536], xt_d[:, 1024:1536])
            nc.sync.dma_start(xt_sb[:, 512:1024], xt_d[:, 512:1024])
            nc.scalar.dma_start(xt_sb[:, 1536:2048], xt_d[:, 1536:2048])
            c3_t = cpool.tile([P, _C3_COLS], bf16, name="c3_t")
            nc.scalar.dma_start(c3_t[:], c3_d[:, :])
            nc.scalar.dma_start(xt_sb[:, 2048:4096], xt_d[:, 2048:4096])

            def xslice(th, e):
                off = 2048 * th + 512 * e
                return xt_sb[:, off : off + 512]

            wq = [c1_t[:, _C1_WQ + e * P : _C1_WQ + (e + 1) * P] for e in range(4)]
            wk = [c1_t[:, _C1_WK + e * P : _C1_WK + (e + 1) * P] for e in range(4)]
            wv = [c3_t[:, _C3_WV + e * P : _C3_WV + (e + 1) * P] for e in range(4)]
            wo_t = c3_t[:, _C3_WO : _C3_WO + 512]
            maskp_t = c3_t[:, _C3_MASK : _C3_MASK + 256]
            ident_t = c3_t[:, _C3_IDENT : _C3_IDENT + 128]
            bq_col = c1_t[:, _C1_BIAS + 0 : _C1_BIAS + 2].bitcast(f32)
            bk_col = c1_t[:, _C1_BIAS + 2 : _C1_BIAS + 4].bitcast(f32)

            # ---- sin/cos tables generated on device ----
            iota_t = cpool.tile([P, L], i32, name="iota_t")
            nc.gpsimd.iota(iota_t[:], pattern=[[1, L]], base=0, channel_multiplier=0)
            h = float(np.pi / (2.0 * L))
            hq = float(np.pi / 2.0)
            sb_bias = cpool.tile([P, 2], f32, name="sb_bias")
            nc.vector.memset(sb_bias[0:D, 0:1], h)
            nc.vector.memset(sb_bias[D:P, 0:1], h + hq)
            nc.vector.memset(sb_bias[0:D, 1:2], h + hq)
            nc.vector.memset(sb_bias[D:P, 1:2], h)
            scb = cpool.tile([P, L], bf16, name="scb")
            nc.scalar.activation(scb[:], iota_t[:], AF.Sin, bias=sb_bias[:, 0:1], scale=h)
            scbf = cpool.tile([P, L], bf16, name="scbf")
            nc.scalar.activation(scbf[:], iota_t[:], AF.Sin, bias=sb_bias[:, 1:2], scale=h)

            # ---- projections (feat-major, bf16) + sin/cos expansion ----
            q_seq = {h2: seqp.tile([P, L], bf16, name=f"q_{h2}") for h2 in "ab"}
            k_seq = {h2: seqp.tile([P, L], bf16, name=f"k_{h2}") for h2 in "ab"}
            v_seq = seqp.tile([P, L], bf16, name="v_pair")
            qp_sb = seqp.tile([P, L], bf16, name="qp_sb")
            kp_sb = seqp.tile([P, L], bf16, name="kp_sb")

            def project(w, th, dst_sl, bias, func, nm):
                ps = big_ps.tile([P, TH], f32, tag="big", name=f"{nm}_ps{th}")
                for e in range(4):
                    nc.tensor.matmul(
                        ps[:], w[e], xslice(th, e), start=(e == 0), stop=(e == 3)
                    )
                if bias is None:
                    nc.scalar.copy(dst_sl, ps[:])
                else:
                    nc.scalar.activation(dst_sl, ps[:], func, bias=bias)

            def expand(dst, src_sb, th):
                """dst[h] rows 0:64 = src_h*sin, rows 64:128 = src_h*cos; all
                DVE input APs share a base partition (outputs may shift)."""
                sl = slice(th * TH, (th + 1) * TH)
                nc.vector.tensor_mul(dst["a"][0:D, sl], src_sb[0:D, sl], scb[0:D, sl])
                nc.vector.tensor_mul(dst["a"][D:P, sl], src_sb[0:D, sl], scbf[0:D, sl])
                nc.vector.tensor_mul(dst["b"][0:D, sl], src_sb[D:P, sl], scbf[D:P, sl])
                nc.vector.tensor_mul(dst["b"][D:P, sl], src_sb[D:P, sl], scb[D:P, sl])

            ktoks = {}

            def kt_piece(th):
                """Hoisted per-chunk token-major k tiles: PE transposes run
                densely inside the projection phase, one DVE copy each."""
                for c in range(4 * th, min(4 * (th + 1), NCHUNK - 1)):
                    cs = slice(c * P, (c + 1) * P)
                    kt_ps = sq_ps.tile([P, 2 * P], bf16, tag="sq", name=f"ktps{c}")
                    nc.tensor.matmul(
                        kt_ps[:, 0:P], k_seq["a"][:, cs], ident_t,
                        is_transpose=True, start=True, stop=False,
                    )
                    nc.tensor.matmul(
                        kt_ps[:, P : 2 * P], k_seq["b"][:, cs], ident_t,
                        is_transpose=True, start=False, stop=True,
                        skip_group_check=True,
                    )
                    ktok = ktokp.tile([P, 2 * P], bf16, tag="ktok", name=f"ktok{c}")
                    nc.vector.tensor_copy(ktok[:], kt_ps[:])
                    ktoks[c] = ktok

            def vt_piece(th):
                """Hoisted vt strips: PE transpose + one strided ACT fill."""
                for c in range(4 * th, 4 * (th + 1)):
                    cs = slice(c * P, (c + 1) * P)
                    vt_c = vt_all[:, c * 130 : (c + 1) * 130]
                    vt_ps = sq_ps.tile([P, P], bf16, tag="sq", name=f"vtps{c}")
                    nc.tensor.matmul(
                        vt_ps[:], v_seq[:, cs], ident_t, is_transpose=True
                    )
                    nc.scalar.copy(
                        vt_c.rearrange("p (a b) -> p a b", a=2, b=65)[:, :, 0:64],
                        vt_ps[:].rearrange("p (a b) -> p a b", a=2, b=64),
                    )

            def proj_piece(th, what):
                sl = slice(th * TH, (th + 1) * TH)
                if what == "q":
                    project(wq, th, qp_sb[:, sl], bq_col[:, 0:1], AF.Relu, "q")
                    expand(q_seq, qp_sb, th)
                elif what == "k":
                    project(wk, th, kp_sb[:, sl], bk_col[:, 0:1], AF.Relu, "k")
                    expand(k_seq, kp_sb, th)
                    kt_piece(th)
                else:
                    project(wv, th, v_seq[:, sl], None, None, "v")
                    vt_piece(th)

            # ---- attention ----
            s_pair = s_ps.tile([P, 2 * (D + 1)], f32, name="s_pair")
            S_prev = [None]

            vt_all = cpool.tile([P, NCHUNK * 2 * (D + 1)], bf16, name="vt_all")
            nc.vector.memset(vt_all[:], 1.0)

            o_sbs = [
                outp.tile([P, 2 * E], bf16, tag="osb", name=f"osb{i}")
                for i in range(2)
            ]

            def attn_chunk(c):
                cs = slice(c * P, (c + 1) * P)
                vt_c = vt_all[:, c * 130 : (c + 1) * 130]
                vt_a, vt_b = vt_c[:, 0:65], vt_c[:, 65:130]
                # masked pair scores
                sc_ps = sq_ps.tile([P, 2 * P], f32, tag="sq", name=f"scps{c}")
                nc.tensor.matmul(
                    sc_ps[:, 0:P], k_seq["a"][:, cs], q_seq["a"][:, cs],
                    start=True, stop=False,
                )
                nc.tensor.matmul(
                    sc_ps[:, P : 2 * P], k_seq["b"][:, cs], q_seq["b"][:, cs],
                    start=False, stop=True, skip_group_check=True,
                )
                b_sb = bp.tile([P, 2 * P], bf16, tag="bsb", name=f"bsb{c}")
                nc.vector.tensor_mul(b_sb[:], sc_ps[:], maskp_t)
                # qkv pair: intra + inter in one bank
                qkv = acc_ps.tile([P, 2 * (D + 1)], f32, tag="acc", name=f"qkv{c}")
                nc.tensor.matmul(
                    qkv[:, 0:65], b_sb[:, 0:P], vt_a, start=True, stop=False
                )
                nc.tensor.matmul(
                    qkv[:, 65:130], b_sb[:, P : 2 * P], vt_b,
                    start=False, stop=(c == 0), skip_group_check=True,
                )
                if c > 0:
                    nc.tensor.matmul(
                        qkv[:, 0:65], q_seq["a"][:, cs], S_prev[0][:, 0:65],
                        start=False, stop=False, skip_group_check=True,
                    )
                    nc.tensor.matmul(
                        qkv[:, 65:130], q_seq["b"][:, cs], S_prev[0][:, 65:130],
                        start=False, stop=True, skip_group_check=True,
                    )
                # state update (skip on last chunk); ktok tiles are hoisted
                if c < NCHUNK - 1:
                    for j in range(2):
                        nc.tensor.matmul(
                            s_pair[:, j * 65 : (j + 1) * 65],
                            ktoks[c][:, j * P : (j + 1) * P],
                            (vt_a, vt_b)[j],
                            start=(c == 0 and j == 0),
                            stop=(c == NCHUNK - 2 and j == 1),
                            skip_group_check=True,
                        )
                    s_new = sp.tile([P, 2 * (D + 1)], bf16, tag="S", name=f"S{c}")
                    nc.scalar.copy(s_new[:], s_pair[:])
                    S_prev[0] = s_new
                # normalize: strided max over both denominators, reciprocal,
                # then ONE broadcast multiply covering both heads.
                r_col = rp.tile([P, 4], f32, tag="r", name=f"r{c}")
                nc.vector.tensor_scalar_max(r_col[:, 0:2], qkv[:, 64:130:65], EPS)
                nc.vector.reciprocal(r_col[:, 2:4], r_col[:, 0:2])
                attn_pair = app.tile([P, P], bf16, tag="ap", name=f"ap{c}")
                nc.vector.tensor_mul(
                    attn_pair[:].rearrange("p (h f) -> p h f", h=2, f=64),
                    qkv[:].rearrange("p (h f) -> p h f", h=2, f=65)[:, :, 0:64],
                    r_col[:, 2:4].unsqueeze(2).broadcast_to([P, 2, 64]),
                )
                # attn^T on the PE, then out projection
                at_ps = sq_ps.tile([P, P], bf16, tag="sq", name=f"atps{c}")
                nc.tensor.matmul(at_ps[:], attn_pair[:], ident_t, is_transpose=True)
                at_sb = atp.tile([P, P], bf16, tag="at", name=f"at{c}")
                nc.scalar.copy(at_sb[:], at_ps[:])
                o_ps = big_ps.tile([P, E], f32, tag="big", name=f"ops{c}")
                nc.tensor.matmul(o_ps[:], at_sb[:], wo_t, start=True, stop=True)
                o_sb = o_sbs[(c // 2) % 2]
                osl = o_sb[:, (c % 2) * E : (c % 2 + 1) * E]
                if c % 2 == 0:
                    nc.vector.tensor_copy(osl, o_ps[:])
                else:
                    nc.scalar.copy(osl, o_ps[:])
                    nc.scalar.dma_start(out_d[:, (c - 1) * E : (c + 1) * E], o_sb[:])

            for what in ("q", "k", "v"):
                proj_piece(0, what)
            attn_chunk(0)
            proj_piece(1, "q")
            attn_chunk(1)
            proj_piece(1, "k")
            attn_chunk(2)
            proj_piece(1, "v")
            for c in range(3, NCHUNK):
                attn_chunk(c)

    nc.compile()
    return nc


def _get_nc():
    if "nc" not in _CACHE:
        _CACHE["nc"] = _build_bass()
    return _CACHE["nc"]


def make_in_maps(query, Wq, bq, Wk, bk, Wv, bv, Wo, bo):
    import ml_dtypes

    f32 = np.float32
    bf16 = ml_dtypes.bfloat16
    query = np.asarray(query, f32)
    x3 = query.reshape(L, BSZ, E)  # faithful torch .view reshape

    Wq, Wk, Wv, Wo = (np.asarray(w, f32) for w in (Wq, Wk, Wv, Wo))
    bq, bk, bv = (np.asarray(b, f32) for b in (bq, bk, bv))

    def wslice_pair(W, p):
        """(128, 512): W rows for the pair, transposed, 4 e-tiles of 128."""
        w = W[P * p : P * (p + 1), :].T  # (512, 128)
        return np.hstack([w[e * P : (e + 1) * P, :] for e in range(4)])

    ident = np.eye(P, dtype=bf16)
    mask = np.triu(np.ones((P, P), f32))
    maskp = np.hstack([mask, mask])

    in_maps = []
    for c in range(N_CORES):
        i, p = divmod(c, 4)

        bias_cols = np.ascontiguousarray(
            np.stack([bq[P * p : P * (p + 1)], bk[P * p : P * (p + 1)]], axis=1)
        ).view(bf16)  # (128, 2) fp32 -> (128, 4) bf16 bits
        c1 = np.hstack(
            [
                bias_cols,
                wslice_pair(Wq, p).astype(bf16),
                wslice_pair(Wk, p).astype(bf16),
            ]
        )
        assert c1.shape == (P, _C1_COLS), c1.shape

        c3 = np.hstack(
            [
                wslice_pair(Wv, p).astype(bf16),
                Wo[:, P * p : P * (p + 1)].T.astype(bf16),  # (128, 512)
                maskp.astype(bf16),
                ident,
            ]
        )
        assert c3.shape == (P, _C3_COLS), c3.shape

        # xt: 8 blocks of (128, 512): block (th, e) at cols 512*(4*th+e)
        xt_full = np.ascontiguousarray(x3[:, i, :].T).astype(bf16)  # (512, 1024)
        blocks = [
            xt_full[128 * e : 128 * (e + 1), 512 * th : 512 * (th + 1)]
            for th in range(2)
            for e in range(4)
        ]
        xt = np.hstack(blocks)
        assert xt.shape == (P, 4096), xt.shape

        in_maps.append(
            dict(
                xt=np.ascontiguousarray(xt),
                c1=np.ascontiguousarray(c1),
                c3=np.ascontiguousarray(c3),
            )
        )
    return in_maps


def assemble(partials, bo, bv, Wo):
    # each partial: (128, 4096) = (p, chunk*512+e) -> (chunk*128+p, e)
    def unpack(arr):
        a = np.asarray(arr, np.float32).reshape(P, NCHUNK, E)
        return a.transpose(1, 0, 2).reshape(L, E)

    out_flat = np.zeros((BSZ * L, E), np.float32)
    out_flat[0::2] = sum(unpack(partials[j]) for j in range(4))
    out_flat[1::2] = sum(unpack(partials[j]) for j in range(4, 8))
    # V-bias passes through the normalized attention additively (exact up to
    # the eps clip): attn(v + bv) = attn(v) + bv, so fold bv @ Wo.T into bo.
    bo_eff = np.asarray(bo, np.float32) + np.asarray(bv, np.float32) @ np.asarray(
        Wo, np.float32
    ).T.astype(np.float32)
    out_flat += bo_eff[None, :]
    return out_flat.reshape(BSZ, L, E)


def run(inputs, trace=False):
    from concourse.bass_utils import run_bass_kernel_spmd

    in_maps = make_in_maps(**inputs)
    nc = _get_nc()
    res = run_bass_kernel_spmd(nc, in_maps, list(range(N_CORES)), trace=trace)
    partials = [r["out"] for r in res.results]
    return assemble(partials, inputs["bo"], inputs["bv"], inputs["Wo"]), res


def kernel(**inputs):
    out, _ = run(inputs, trace=False)
    return out


# revision 23
# speedup vs baseline: 1.3049x; 1.3049x over previous
"""Cosformer attention (causal linear attention with cos reweighting) on 8
Trainium2 NeuronCores.

Sharding: n = bsz*heads = 16 sequences -> 2 per core. Core c handles batch-half
i = c//4 and head-pair p = c%4 (heads 2p, 2p+1). Fully data/head parallel; the
only cross-core interaction is the host-side sum of output-projection partials.

v4 design notes:
  - All DMA images bf16; raw (non-duplicated) pair projections (24 matmuls);
    [sin;cos] feature expansion via DVE elementwise ops against two on-device
    tables (scb=[sin;cos] rows, scbf=[cos;sin] rows so inputs always share a
    base partition; outputs may be partition-shifted).
  - Pair-merged single-bank PSUM tiles: scores (128,256), qkv (128,130),
    S-state (128,130, persistent): one start=True clears the bank, the other
    region writes use start=False (has_written=0 -> overwrite).
  - Normalize: one strided ts_max over both denominators + two ts_divide
    (per-partition divisor) straight out of PSUM.
  - attn^T for the output projection comes from an SBUF->SBUF DMA transpose
    on the idle sync ring (no PE transpose, no PSUM, no ACT copy).
  - th1 projections are interleaved between attention chunks 0-3 to keep the
    PE dense (HAM stays at full clock) and avoid a serial projection wall.
  - Output (128, 4096) bf16 chunk-major; 4 HWDGE stores (2 chunks each).
"""

import os
import sys

import numpy as np

for _p in ("/opt/trn_rl_repo", "/root/.axon_site/_ro/trn_rl_repo"):
    if os.path.isdir(_p) and _p not in sys.path:
        sys.path.insert(0, _p)

N_HEAD = 8
E = 512
L = 1024  # sequence length per batch-half
BSZ = 2
D = 64  # head dim
P = 128  # partition/chunk/pair-feature size
NCHUNK = L // P
EPS = 1e-6
N_CORES = 8
TH = 512  # token-half width for projections

# c1: [biases as fp32-bits (4 bf16 cols: bq_pair, bk_pair) | wq_pair | wk_pair]
# loaded as two micro-packs: c1[:, 0:516] (bias+wq) on sync, c1[:, 516:1028]
# (wk) on scalar, so the first projection can start as early as possible.
_C1_BIAS = 0
_C1_WQ = 4
_C1_WK = 516
_C1_COLS = 1028
# c3: [wv (512) | wo (512) | mask_pair (256) | ident (128)]
_C3_WV = 0
_C3_WO = 512
_C3_MASK = 1024
_C3_IDENT = 1280
_C3_COLS = 1408

_CACHE = {}


def _build_bass():
    import concourse.bass as bass
    import concourse.tile as tile
    from concourse import bacc, mybir
    from contextlib import ExitStack

    f32 = mybir.dt.float32
    i32 = mybir.dt.int32
    bf16 = mybir.dt.bfloat16
    AF = mybir.ActivationFunctionType
    ALU = mybir.AluOpType

    nc = bacc.Bacc("TRN2", target_bir_lowering=False, debug=False)

    xt_d = nc.dram_tensor("xt", [P, 4096], bf16, kind="ExternalInput")
    c1_d = nc.dram_tensor("c1", [P, _C1_COLS], bf16, kind="ExternalInput")
    c3_d = nc.dram_tensor("c3", [P, _C3_COLS], bf16, kind="ExternalInput")
    out_d = nc.dram_tensor("out", [P, 4096], bf16, kind="ExternalOutput")

    with tile.TileContext(nc) as tc:
        with ExitStack() as ctx:
            ep = ctx.enter_context
            cpool = ep(tc.tile_pool(name="const", bufs=1))
            seqp = ep(tc.tile_pool(name="seq", bufs=1))
            ktokp = ep(tc.tile_pool(name="ktok", bufs=8))
            bp = ep(tc.tile_pool(name="bsb", bufs=4))
            sp = ep(tc.tile_pool(name="state", bufs=4))
            app = ep(tc.tile_pool(name="apair", bufs=2))
            atp = ep(tc.tile_pool(name="attnT", bufs=2))
            outp = ep(tc.tile_pool(name="outsb", bufs=2))
            rp = ep(tc.tile_pool(name="rcol", bufs=4))
            big_ps = ep(tc.tile_pool(name="bigps", bufs=2, space="PSUM"))
            sq_ps = ep(tc.tile_pool(name="sqps", bufs=3, space="PSUM"))
            acc_ps = ep(tc.tile_pool(name="accps", bufs=2, space="PSUM"))
            s_ps = ep(tc.tile_pool(name="sps", bufs=1, space="PSUM"))

            # ---- loads: two HWDGE rings, first-needed bytes first.
            # sync ring then goes quiet early so the attn-phase at/kt
            # DMA-transposes never queue behind bulk traffic; all output
            # stores go on the scalar ring.
            c1_t = cpool.tile([P, _C1_COLS], bf16, name="c1_t")
            xt_sb = cpool.tile([P, 4096], bf16, name="xt_sb")
            nc.sync.dma_start(c1_t[:, 0:516], c1_d[:, 0:516])
            nc.scalar.dma_start(c1_t[:, 516:1028], c1_d[:, 516:1028])
            nc.sync.dma_start(xt_sb[:, 0:512], xt_d[:, 0:512])
            nc.scalar.dma_start(xt_sb[:, 1024:1536], xt_d[:, 1024:1536])
            nc.sync.dma_start(xt_sb[:, 512:1024], xt_d[:, 512:1024])
            nc.scalar.dma_start(xt_sb[:, 1536:2048], xt_d[:, 1536:2048])
            c3_t = cpool.tile([P, _C3_COLS], bf16, name="c3_t")
            nc.scalar.dma_start(c3_t[:], c3_d[:, :])
            nc.scalar.dma_start(xt_sb[:, 2048:4096], xt_d[:, 2048:4096])

            def xslice(th, e):
                off = 2048 * th + 512 * e
                return xt_sb[:, off : off + 512]

            wq = [c1_t[:, _C1_WQ + e * P : _C1_WQ + (e + 1) * P] for e in range(4)]
            wk = [c1_t[:, _C1_WK + e * P : _C1_WK + (e + 1) * P] for e in range(4)]
            wv = [c3_t[:, _C3_WV + e * P : _C3_WV + (e + 1) * P] for e in range(4)]
            wo_t = c3_t[:, _C3_WO : _C3_WO + 512]
            maskp_t = c3_t[:, _C3_MASK : _C3_MASK + 256]
            ident_t = c3_t[:, _C3_IDENT : _C3_IDENT + 128]
            bq_col = c1_t[:, _C1_BIAS + 0 : _C1_BIAS + 2].bitcast(f32)
            bk_col = c1_t[:, _C1_BIAS + 2 : _C1_BIAS + 4].bitcast(f32)

            # ---- sin/cos tables generated on device ----
            iota_t = cpool.tile([P, L], i32, name="iota_t")
            nc.gpsimd.iota(iota_t[:], pattern=[[1, L]], base=0, channel_multiplier=0)
            h = float(np.pi / (2.0 * L))
            hq = float(np.pi / 2.0)
            sb_bias = cpool.tile([P, 2], f32, name="sb_bias")
            nc.vector.memset(sb_bias[0:D, 0:1], h)
            nc.vector.memset(sb_bias[D:P, 0:1], h + hq)
            nc.vector.memset(sb_bias[0:D, 1:2], h + hq)
            nc.vector.memset(sb_bias[D:P, 1:2], h)
            scb = cpool.tile([P, L], bf16, name="scb")
            nc.scalar.activation(scb[:], iota_t[:], AF.Sin, bias=sb_bias[:, 0:1], scale=h)
            scbf = cpool.tile([P, L], bf16, name="scbf")
            nc.scalar.activation(scbf[:], iota_t[:], AF.Sin, bias=sb_bias[:, 1:2], scale=h)

            # ---- projections (feat-major, bf16) + sin/cos expansion ----
            q_seq = {h2: seqp.tile([P, L], bf16, name=f"q_{h2}") for h2 in "ab"}
            k_seq = {h2: seqp.tile([P, L], bf16, name=f"k_{h2}") for h2 in "ab"}
            v_seq = seqp.tile([P, L], bf16, name="v_pair")
            qp_sb = seqp.tile([P, L], bf16, name="qp_sb")
            kp_sb = seqp.tile([P, L], bf16, name="kp_sb")

            def project(w, th, dst_sl, bias, func, nm):
                ps = big_ps.tile([P, TH], f32, tag="big", name=f"{nm}_ps{th}")
                for e in range(4):
                    nc.tensor.matmul(
                        ps[:], w[e], xslice(th, e), start=(e == 0), stop=(e == 3)
                    )
                if bias is None:
                    nc.scalar.copy(dst_sl, ps[:])
                else:
                    nc.scalar.activation(dst_sl, ps[:], func, bias=bias)

            def expand(dst, src_sb, th):
                """dst[h] rows 0:64 = src_h*sin, rows 64:128 = src_h*cos; all
                DVE input APs share a base partition (outputs may shift)."""
                sl = slice(th * TH, (th + 1) * TH)
                nc.vector.tensor_mul(dst["a"][0:D, sl], src_sb[0:D, sl], scb[0:D, sl])
                nc.vector.tensor_mul(dst["a"][D:P, sl], src_sb[0:D, sl], scbf[0:D, sl])
                nc.vector.tensor_mul(dst["b"][0:D, sl], src_sb[D:P, sl], scbf[D:P, sl])
                nc.vector.tensor_mul(dst["b"][D:P, sl], src_sb[D:P, sl], scb[D:P, sl])

            def proj_piece(th, what):
                sl = slice(th * TH, (th + 1) * TH)
                if what == "q":
                    project(wq, th, qp_sb[:, sl], bq_col[:, 0:1], AF.Relu, "q")
                    expand(q_seq, qp_sb, th)
                elif what == "k":
                    project(wk, th, kp_sb[:, sl], bk_col[:, 0:1], AF.Relu, "k")
                    expand(k_seq, kp_sb, th)
                else:
                    project(wv, th, v_seq[:, sl], None, None, "v")

            # ---- attention ----
            s_pair = s_ps.tile([P, 2 * (D + 1)], f32, name="s_pair")
            S_prev = [None]

            vt_all = cpool.tile([P, NCHUNK * 2 * (D + 1)], bf16, name="vt_all")
            nc.vector.memset(vt_all[:], 1.0)

            o_sbs = [
                outp.tile([P, 2 * E], bf16, tag="osb", name=f"osb{i}")
                for i in range(2)
            ]

            def attn_chunk(c):
                cs = slice(c * P, (c + 1) * P)
                vt_c = vt_all[:, c * 130 : (c + 1) * 130]
                vt_a, vt_b = vt_c[:, 0:65], vt_c[:, 65:130]
                vt_ps = sq_ps.tile([P, P], bf16, tag="sq", name=f"vtps{c}")
                nc.tensor.matmul(vt_ps[:], v_seq[:, cs], ident_t, is_transpose=True)
                nc.scalar.copy(
                    vt_c.rearrange("p (a b) -> p a b", a=2, b=65)[:, :, 0:64],
                    vt_ps[:].rearrange("p (a b) -> p a b", a=2, b=64),
                )
                # masked pair scores
                sc_ps = sq_ps.tile([P, 2 * P], f32, tag="sq", name=f"scps{c}")
                nc.tensor.matmul(
                    sc_ps[:, 0:P], k_seq["a"][:, cs], q_seq["a"][:, cs],
                    start=True, stop=False,
                )
                nc.tensor.matmul(
                    sc_ps[:, P : 2 * P], k_seq["b"][:, cs], q_seq["b"][:, cs],
                    start=False, stop=True, skip_group_check=True,
                )
                b_sb = bp.tile([P, 2 * P], bf16, tag="bsb", name=f"bsb{c}")
                nc.vector.tensor_mul(b_sb[:], sc_ps[:], maskp_t)
                # qkv pair: intra + inter in one bank
                qkv = acc_ps.tile([P, 2 * (D + 1)], f32, tag="acc", name=f"qkv{c}")
                nc.tensor.matmul(
                    qkv[:, 0:65], b_sb[:, 0:P], vt_a, start=True, stop=False
                )
                nc.tensor.matmul(
                    qkv[:, 65:130], b_sb[:, P : 2 * P], vt_b,
                    start=False, stop=(c == 0), skip_group_check=True,
                )
                if c > 0:
                    nc.tensor.matmul(
                        qkv[:, 0:65], q_seq["a"][:, cs], S_prev[0][:, 0:65],
                        start=False, stop=False, skip_group_check=True,
                    )
                    nc.tensor.matmul(
                        qkv[:, 65:130], q_seq["b"][:, cs], S_prev[0][:, 65:130],
                        start=False, stop=True, skip_group_check=True,
                    )
                # state update (skip on last chunk): both heads' token-major k
                # tiles land in one bf16 PSUM bank, then one DVE copy.
                if c < NCHUNK - 1:
                    kt_ps = sq_ps.tile([P, 2 * P], bf16, tag="sq", name=f"ktps{c}")
                    nc.tensor.matmul(
                        kt_ps[:, 0:P], k_seq["a"][:, cs], ident_t,
                        is_transpose=True, start=True, stop=False,
                    )
                    nc.tensor.matmul(
                        kt_ps[:, P : 2 * P], k_seq["b"][:, cs], ident_t,
                        is_transpose=True, start=False, stop=True,
                        skip_group_check=True,
                    )
                    ktok = ktokp.tile([P, 2 * P], bf16, tag="ktok", name=f"ktok{c}")
                    nc.vector.tensor_copy(ktok[:], kt_ps[:])
                    for j in range(2):
                        nc.tensor.matmul(
                            s_pair[:, j * 65 : (j + 1) * 65],
                            ktok[:, j * P : (j + 1) * P],
                            (vt_a, vt_b)[j],
                            start=(c == 0 and j == 0),
                            stop=(c == NCHUNK - 2 and j == 1),
                            skip_group_check=True,
                        )
                    s_new = sp.tile([P, 2 * (D + 1)], bf16, tag="S", name=f"S{c}")
                    nc.scalar.copy(s_new[:], s_pair[:])
                    S_prev[0] = s_new
                # normalize: strided max over both denominators, reciprocal,
                # then ONE broadcast multiply covering both heads.
                r_col = rp.tile([P, 4], f32, tag="r", name=f"r{c}")
                nc.vector.tensor_scalar_max(r_col[:, 0:2], qkv[:, 64:130:65], EPS)
                nc.vector.reciprocal(r_col[:, 2:4], r_col[:, 0:2])
                attn_pair = app.tile([P, P], bf16, tag="ap", name=f"ap{c}")
                nc.vector.tensor_mul(
                    attn_pair[:].rearrange("p (h f) -> p h f", h=2, f=64),
                    qkv[:].rearrange("p (h f) -> p h f", h=2, f=65)[:, :, 0:64],
                    r_col[:, 2:4].unsqueeze(2).broadcast_to([P, 2, 64]),
                )
                # attn^T on the PE, then out projection
                at_ps = acc_ps.tile([P, P], bf16, tag="acc", name=f"atps{c}")
                nc.tensor.matmul(at_ps[:], attn_pair[:], ident_t, is_transpose=True)
                at_sb = atp.tile([P, P], bf16, tag="at", name=f"at{c}")
                nc.scalar.copy(at_sb[:], at_ps[:])
                o_ps = big_ps.tile([P, E], f32, tag="big", name=f"ops{c}")
                nc.tensor.matmul(o_ps[:], at_sb[:], wo_t, start=True, stop=True)
                o_sb = o_sbs[(c // 2) % 2]
                osl = o_sb[:, (c % 2) * E : (c % 2 + 1) * E]
                if c % 2 == 0:
                    nc.vector.tensor_copy(osl, o_ps[:])
                else:
                    nc.scalar.copy(osl, o_ps[:])
                    nc.scalar.dma_start(out_d[:, (c - 1) * E : (c + 1) * E], o_sb[:])

            for what in ("q", "k", "v"):
                proj_piece(0, what)
            attn_chunk(0)
            proj_piece(1, "q")
            attn_chunk(1)
            proj_piece(1, "k")
            attn_chunk(2)
            proj_piece(1, "v")
            for c in range(3, NCHUNK):
                attn_chunk(c)

    nc.compile()
    return nc


def _get_nc():
    if "nc" not in _CACHE:
        _CACHE["nc"] = _build_bass()
    return _CACHE["nc"]


def make_in_maps(query, Wq, bq, Wk, bk, Wv, bv, Wo, bo):
    import ml_dtypes

    f32 = np.float32
    bf16 = ml_dtypes.bfloat16
    query = np.asarray(query, f32)
    x3 = query.reshape(L, BSZ, E)  # faithful torch .view reshape

    Wq, Wk, Wv, Wo = (np.asarray(w, f32) for w in (Wq, Wk, Wv, Wo))
    bq, bk, bv = (np.asarray(b, f32) for b in (bq, bk, bv))

    def wslice_pair(W, p):
        """(128, 512): W rows for the pair, transposed, 4 e-tiles of 128."""
        w = W[P * p : P * (p + 1), :].T  # (512, 128)
        return np.hstack([w[e * P : (e + 1) * P, :] for e in range(4)])

    ident = np.eye(P, dtype=bf16)
    mask = np.triu(np.ones((P, P), f32))
    maskp = np.hstack([mask, mask])

    in_maps = []
    for c in range(N_CORES):
        i, p = divmod(c, 4)

        bias_cols = np.ascontiguousarray(
            np.stack([bq[P * p : P * (p + 1)], bk[P * p : P * (p + 1)]], axis=1)
        ).view(bf16)  # (128, 2) fp32 -> (128, 4) bf16 bits
        c1 = np.hstack(
            [
                bias_cols,
                wslice_pair(Wq, p).astype(bf16),
                wslice_pair(Wk, p).astype(bf16),
            ]
        )
        assert c1.shape == (P, _C1_COLS), c1.shape

        c3 = np.hstack(
            [
                wslice_pair(Wv, p).astype(bf16),
                Wo[:, P * p : P * (p + 1)].T.astype(bf16),  # (128, 512)
                maskp.astype(bf16),
                ident,
            ]
        )
        assert c3.shape == (P, _C3_COLS), c3.shape

        # xt: 8 blocks of (128, 512): block (th, e) at cols 512*(4*th+e)
        xt_full = np.ascontiguousarray(x3[:, i, :].T).astype(bf16)  # (512, 1024)
        blocks = [
            xt_full[128 * e : 128 * (e + 1), 512 * th : 512 * (th + 1)]
            for th in range(2)
            for e in range(4)
        ]
        xt = np.hstack(blocks)
        assert xt.shape == (P, 4096), xt.shape

        in_maps.append(
            dict(
                xt=np.ascontiguousarray(xt),
                c1=np.ascontiguousarray(c1),
                c3=np.ascontiguousarray(c3),
            )
        )
    return in_maps


def assemble(partials, bo, bv, Wo):
    # each partial: (128, 4096) = (p, chunk*512+e) -> (chunk*128+p, e)
    def unpack(arr):
        a = np.asarray(arr, np.float32).reshape(P, NCHUNK, E)
        return a.transpose(1, 0, 2).reshape(L, E)

    out_flat = np.zeros((BSZ * L, E), np.float32)
    out_flat[0::2] = sum(unpack(partials[j]) for j in range(4))
    out_flat[1::2] = sum(unpack(partials[j]) for j in range(4, 8))
    # V-bias passes through the normalized attention additively (exact up to
    # the eps clip): attn(v + bv) = attn(v) + bv, so fold bv @ Wo.T into bo.
    bo_eff = np.asarray(bo, np.float32) + np.asarray(bv, np.float32) @ np.asarray(
        Wo, np.float32
    ).T.astype(np.float32)
    out_flat += bo_eff[None, :]
    return out_flat.reshape(BSZ, L, E)


def run(inputs, trace=False):
    from concourse.bass_utils import run_bass_kernel_spmd

    in_maps = make_in_maps(**inputs)
    nc = _get_nc()
    res = run_bass_kernel_spmd(nc, in_maps, list(range(N_CORES)), trace=trace)
    partials = [r["out"] for r in res.results]
    return assemble(partials, inputs["bo"], inputs["bv"], inputs["Wo"]), res


def kernel(**inputs):
    out, _ = run(inputs, trace=False)
    return out
